# revision 22
# baseline (speedup 1.0000x reference)
"""Trainium2 Bass kernel for nn_ContextualAttention (sparse_attention).

Contract: kernel(**inputs) takes FULL numpy inputs and returns the FULL
[2, 256, 48, 48] float32 output. Internally shards across 8 NeuronCores as
(batch b in {0,1}) x (side l/r) x (position-half in {0,1}).

Per-core device work for unit (b, side), half h:
  scores_T[p, l] = sum_{ki,kj,c} mid[c, y+ki-1, x+kj-1] * feat[c, ly+ki-1, lx+kj-1]
    (contraction tiled as 9 spatial offsets x 2 channel-halves of 128; the
     shifted windows are contiguous 1-D APs into 24-wide images; the mid-side
     x-edge wrap is handled by three host-sent variants with the contaminated
     column zeroed; the feat-side wrap is handled with strided 2-D APs that
     skip the contaminated filter column, so only ONE fp copy is DMAed)
  scores_T *= inv_denom[l]  (host-computed feature-patch L2 norms; drained
     from PSUM on the Pool engine so the DVE softmax chain never blocks it)
  attn_T = softmax over l (free axis), scale 10
  attn   = transpose(attn_T)            (PE transpose, 128-blocks)
  out[cf, p] = sum_l rawT[l, cf] * attn[l, p]   (cf = c*16 + i*4 + j)
    (PSUM drained round-robin over DVE/Pool/ACT into bf16 staging, DMAed
     out as bf16 and widened on host)

Host: downsample, build mh wrap-variants + rawT via as_strided, overlap-add
the transpose-conv contributions, cosine blend.
"""

import sys

for _p in ("/opt/trn_rl_repo", "/root/.axon_site/_ro/trn_rl_repo"):
    if _p not in sys.path:
        sys.path.append(_p)

import numpy as np
import ml_dtypes

BF16 = ml_dtypes.bfloat16

B, C, H, W = 2, 256, 48, 48
HD = WD = 24          # downsampled spatial
L = HD * WD           # 576 filter positions
PH = L // 2           # 288 positions per core (half)
CF = C * 16           # 4096 reconstruction features (c, i, j)
EPS_SUM = 2304 * 1e-4  # sum_k (f^2 + eps) = sumsq + K*eps
SCALE = 10.0
MHW = 14 * 24 + 2     # mh row length incl 1-elem zero guards
FPW = 26 * 24 + 2     # fp row length incl guards

_CACHED = {}


def _build_nc(reps=1):
    from concourse import bacc, mybir
    from concourse.dt import dt
    from concourse.tile import TileContext

    f32 = dt.float32
    f32r = dt.float32r
    bf16 = dt.bfloat16

    nc = bacc.Bacc("TRN2", target_bir_lowering=False, debug=False,
                   num_devices=8)
    mh_d = nc.declare_dram_parameter("mh3", [3 * C, MHW], bf16, isOutput=False)
    fp_d = nc.declare_dram_parameter("fp1", [C, FPW], bf16, isOutput=False)
    rawT_d = nc.declare_dram_parameter("rawT", [L, CF], bf16, isOutput=False)
    id_d = nc.declare_dram_parameter("ident", [128, 128], bf16, isOutput=False)
    iv_d = nc.declare_dram_parameter("dinv", [128, L], f32, isOutput=False)
    out_d = nc.declare_dram_parameter("out", [CF, PH], bf16, isOutput=True)

    AX = mybir.AxisListType.X
    OP = mybir.AluOpType
    AF = mybir.ActivationFunctionType

    # l-tiles for the 576-long filter axis: 4x128 + 64
    LT = [(0, 128), (128, 128), (256, 128), (384, 128), (512, 64)]

    with TileContext(nc) as tc:
        with (
            tc.tile_pool(name="persist", bufs=1) as pp,
            tc.tile_pool(name="stats", bufs=4) as sp,
            tc.tile_pool(name="ps_score", bufs=2, space="PSUM") as ps_s,
            tc.tile_pool(name="ps_tr", bufs=3, space="PSUM") as ps_t,
            tc.tile_pool(name="ps_out", bufs=3, space="PSUM") as ps_o,
            tc.tile_pool(name="ostage", bufs=4) as op_,
        ):
          def ecopy(i, out_ap, in_ap):
              # alternate PSUM drains over DVE/ACT (Pool cannot touch PSUM)
              if i % 2 == 0:
                  nc.vector.tensor_copy(out_ap, in_ap)
              else:
                  nc.scalar.copy(out_ap, in_ap)

          for _rep in range(reps):
              # ---- persistent SBUF tensors + input DMAs ----
              # channel-pairs batched into single wide tiles: one DMA each
              mhB = [pp.tile([128, 2 * MHW], bf16, tag=f"mhB{v}", name=f"mhB{v}")
                     for v in range(3)]
              fpB = pp.tile([128, 2 * FPW], bf16, tag="fpB", name="fpB")
              mh = [[mhB[v][:, ch * MHW:(ch + 1) * MHW] for ch in range(2)]
                    for v in range(3)]
              fp = [fpB[:, ch * FPW:(ch + 1) * FPW] for ch in range(2)]
              rawT = [pp.tile([128, CF], bf16, tag=f"rawT{i}", name=f"rawT{i}")
                      for i in range(5)]
              ident = pp.tile([128, 128], bf16, tag="ident", name="ident")
              attnT = [pp.tile([96, L], f32, tag=f"attnT{i}", name=f"attnT{i}")
                       for i in range(3)]
              attnTb = [pp.tile([96, L], bf16, tag=f"attnTb{i}", name=f"attnTb{i}")
                        for i in range(3)]
              attn = [pp.tile([128, PH], bf16, tag=f"attn{i}", name=f"attn{i}")
                      for i in range(5)]
              dinv = pp.tile([128, L], f32, tag="dinv", name="dinv")

              # DMAs in consumption order: invd + fp + mh (scores) first,
              # then ident + rawT (recon). Channel pairs ride one descriptor
              # set each to amortize the ~625ns HWDGE fixed cost per DMA.
              nc.sync.dma_start(
                  fpB[:, :].rearrange("p (b c) -> p b c", b=2),
                  fp_d[:, :].rearrange("(b p) c -> p b c", b=2))
              for v in (1, 0, 2):
                  nc.sync.dma_start(
                      mhB[v][:, :].rearrange("p (b c) -> p b c", b=2),
                      mh_d[v * C:(v + 1) * C, :].rearrange(
                          "(b p) c -> p b c", b=2))
              nc.sync.dma_start(dinv[:, :], iv_d[:, :])
              nc.sync.dma_start(ident[:, :], id_d[:, :])
              for lt, (l0, lsz) in enumerate(LT):
                  nc.sync.dma_start(rawT[lt][0:lsz, :], rawT_d[l0:l0 + lsz, :])

              # ---- PE p-state warm-up: ~3.2us of dummy matmuls so the
              # tensor engine is at full clock when real work arrives ----
              warm = pp.tile([128, 512], bf16, tag="warm", name="warm")
              nc.vector.memset(warm[:, :], 0.0)
              for w in range(10):
                  wps = ps_s.tile([96, PH], f32, tag="ps", name="wps")
                  nc.tensor.matmul(wps[0:64, :], warm[:, 0:64],
                                   warm[:, 0:PH], start=True, stop=True)

              # ---- scores + softmax, one 96-position tile at a time ----
              esums = []

              def transposes(t):
                  # attn_T -> attn [l, p]; emitted right after finalize(t) so
                  # earlier tiles' transposes+drains hide under later scores
                  for lt, (l0, lsz) in enumerate(LT):
                      tr = ps_t.tile([128, 96], bf16, tag="tr", name="tr")
                      nc.tensor.transpose(tr[0:lsz, :],
                                          attnTb[t][:, l0:l0 + lsz],
                                          ident[0:96, 0:96])
                      ecopy(t * 5 + lt,
                            attn[lt][0:lsz, t * 96:(t + 1) * 96], tr[0:lsz, :])

              def finalize(t):
                  rinv = sp.tile([96, 1], f32, tag="rinv", name="rinv")
                  nc.vector.reciprocal(rinv[:, :], esums[t][:, :])
                  if t < 2:
                      nc.gpsimd.tensor_scalar_mul(attnTb[t][:, :],
                                                  attnT[t][:, :], rinv[:, :])
                  else:
                      # critical tail: split halves over the two fast engines
                      nc.vector.tensor_scalar_mul(attnTb[t][:, 0:PH],
                                                  attnT[t][:, 0:PH],
                                                  rinv[:, :])
                      nc.scalar.activation(attnTb[t][:, PH:L],
                                           attnT[t][:, PH:L], AF.Copy,
                                           scale=rinv[:, :])

              for t in range(3):
                  rmh = sp.tile([96, 2], f32, tag="rmh", name="rmh")
                  for lh in range(2):
                      ps = ps_s.tile([96, PH], f32, tag="ps", name="ps")
                      k = 0
                      for kj in (1, 0, 2):
                          for ch in range(2):
                              for ki in range(3):
                                  lo = 1 + (4 * t + ki) * 24 + kj - 1
                                  lhsT = mh[kj][ch][:, lo:lo + 96]
                                  if kj == 1:
                                      ro = 1 + (12 * lh + ki) * 24
                                      rhs = fp[ch][:, ro:ro + PH]
                                      o = ps[:, :]
                                  elif kj == 0:
                                      # feat window x = lx-1; filter col lx=0
                                      # contaminated -> skip it (2-D AP)
                                      ro = (12 * lh + ki) * 24
                                      rhs = fp[ch][:, ro:ro + PH].rearrange(
                                          "p (r e) -> p r e", e=24)[:, :, 1:24]
                                      o = ps[:, :].rearrange(
                                          "p (r e) -> p r e", e=24)[:, :, 1:24]
                                  else:
                                      # feat window x = lx+1; skip lx=23
                                      ro = 2 + (12 * lh + ki) * 24
                                      rhs = fp[ch][:, ro:ro + PH].rearrange(
                                          "p (r e) -> p r e", e=24)[:, :, 0:23]
                                      o = ps[:, :].rearrange(
                                          "p (r e) -> p r e", e=24)[:, :, 0:23]
                                  nc.tensor.matmul(o, lhsT, rhs,
                                                   start=(k == 0), stop=(k == 17))
                                  k += 1
                      # normalize by feature-patch norms while draining PSUM
                      # (DVE; Pool cannot access PSUM on hardware)
                      nc.vector.tensor_mul(attnT[t][:, lh * PH:(lh + 1) * PH],
                                           ps[:, :],
                                           dinv[0:96, lh * PH:(lh + 1) * PH])
                      nc.vector.tensor_reduce(
                          rmh[:, lh:lh + 1],
                          attnT[t][:, lh * PH:(lh + 1) * PH], AX, OP.max)
                  # max/bias/final-scale live on Pool (SBUF-only ops) so the
                  # DVE drain stream never blocks behind them; recip stays on
                  # DVE but is emitted one t late (exp has long finished)
                  rm = sp.tile([96, 1], f32, tag="rm", name="rm")
                  nbias = sp.tile([96, 1], f32, tag="nbias", name="nbias")
                  esum = sp.tile([96, 1], f32, tag="esum", name="esum")
                  nc.vector.tensor_reduce(rm[:, :], rmh[:, :], AX, OP.max)
                  nc.gpsimd.tensor_scalar_mul(nbias[:, :], rm[:, :], -SCALE)
                  nc.scalar.activation(attnT[t][:, :], attnT[t][:, :], AF.Exp,
                                       bias=nbias[:, :], scale=SCALE,
                                       accum_out=esum[:, :])
                  esums.append(esum)
                  if t > 0:
                      finalize(t - 1)
                      transposes(t - 1)
              finalize(2)
              transposes(2)

              # ---- reconstruction: out[cf, p] = sum_l rawT[l, cf] attn[l, p] ----
              # outputs staged four cf-blocks wide so one DMA covers 512 rows
              QUAD = 4
              ost = None
              for cf in range(CF // 128):
                  po = ps_o.tile([128, PH], f32, tag="po", name="po")
                  for lt, (l0, lsz) in enumerate(LT):
                      nc.tensor.matmul(
                          po[:, :],
                          rawT[lt][0:lsz, cf * 128:(cf + 1) * 128],
                          attn[lt][0:lsz, :],
                          start=(lt == 0), stop=(lt == 4))
                  grp = 2 if cf >= 28 else QUAD
                  q = cf % grp
                  if q == 0:
                      ost = op_.tile([128, QUAD * PH], bf16, tag="ost",
                                     name="ost")
                  ecopy(cf, ost[:, q * PH:(q + 1) * PH], po[:, :])
                  if q == grp - 1:
                      nc.sync.dma_start(
                          out_d[(cf - grp + 1) * 128:(cf + 1) * 128, :]
                          .rearrange("(b p) c -> p b c", b=grp),
                          ost[:, 0:grp * PH].rearrange(
                              "p (b c) -> p b c", b=grp))

    nc.compile()
    return nc


def _variants(img, rows):
    """img: [C, rows, 24] -> [3, C, rows*24+2] with 1-elem zero guards and the
    wrap-contaminated column zeroed per kj variant (kj=0: col 23, kj=2: col 0).
    """
    out = np.zeros((3, C, rows * 24 + 2), np.float32)
    vl = img.copy(); vl[:, :, 23] = 0.0
    vr = img.copy(); vr[:, :, 0] = 0.0
    for v, arr in enumerate((vl, img, vr)):
        out[v, :, 1:1 + rows * 24] = arr.reshape(C, rows * 24)
    return out


def _prep_inputs(inputs):
    """Build the 8 per-core input maps from the full problem inputs."""
    left = np.asarray(inputs["left"], dtype=np.float32)
    right = np.asarray(inputs["right"], dtype=np.float32)
    mid = np.asarray(inputs["mid"], dtype=np.float32)
    sl = np.asarray(inputs["shortcut_l"], dtype=np.float32)
    sr = np.asarray(inputs["shortcut_r"], dtype=np.float32)

    m_ds = mid[:, :, ::2, ::2]
    f_ds = [left[:, :, ::2, ::2], right[:, :, ::2, ::2]]

    # mh: rows y in [-1, 12] (h=0) / [11, 24] (h=1), zero at out-of-range
    mh3 = np.zeros((B, 2, 3, C, MHW), np.float32)
    for b in range(B):
        for h in range(2):
            m14 = np.zeros((C, 14, 24), np.float32)
            if h == 0:
                m14[:, 1:14] = m_ds[b, :, 0:13]
            else:
                m14[:, 0:13] = m_ds[b, :, 11:24]
            mh3[b, h] = _variants(m14, 14).reshape(3 * C, MHW).reshape(
                3, C, MHW)
    # fp: rows y in [-1, 24]; single base copy (wrap handled by strided APs)
    fp1 = np.zeros((B, 2, C, FPW), np.float32)
    invd = np.zeros((B, 2, 1, L), np.float32)
    for b in range(B):
        for side in range(2):
            f26 = np.zeros((C, 26, 24), np.float32)
            f26[:, 1:25] = f_ds[side][b]
            fp1[b, side, :, 1:1 + 26 * 24] = f26.reshape(C, 26 * 24)
            # host inv_denom: 3x3 window sums of per-pixel channel sumsq
            s = np.zeros((26, 26), np.float32)
            s[1:25, 1:25] = (f_ds[side][b] ** 2).sum(axis=0)
            d2 = np.zeros((24, 24), np.float32)
            for ki in range(3):
                for kj in range(3):
                    d2 += s[ki:ki + 24, kj:kj + 24]
            invd[b, side] = (1.0 / np.sqrt(d2 + EPS_SUM)).reshape(1, L)

    def raw_t(s):  # [C,48,48] -> [576, 4096] (l=(y,x), cf=(c,i,j))
        p = np.zeros((C, 50, 50), np.float32)
        p[:, 1:49, 1:49] = s
        st = p.strides
        v = np.lib.stride_tricks.as_strided(
            p, shape=(24, 24, C, 4, 4),
            strides=(2 * st[1], 2 * st[2], st[0], st[1], st[2]))
        return np.ascontiguousarray(v).reshape(L, CF)

    raws = [[raw_t(sl[b]), raw_t(sr[b])] for b in range(B)]
    ident = np.eye(128, dtype=np.float32)

    in_maps = []
    for core in range(8):
        b, side, h = core >> 2, (core >> 1) & 1, core & 1
        in_maps.append({
            "mh3": mh3[b, h].reshape(3 * C, MHW).astype(BF16),
            "fp1": fp1[b, side].astype(BF16),
            "rawT": raws[b][side].astype(BF16),
            "ident": ident.astype(BF16),
            "dinv": np.broadcast_to(invd[b, side], (128, L)).copy(),
        })
    return in_maps


def _postprocess(results):
    """results: list of 8 dicts with 'out' [4096, 288] bf16 -> full output."""
    y = np.zeros((B, 2, C, 48, 48), np.float32)
    for b in range(B):
        for side in range(2):
            feat = np.concatenate(
                [np.asarray(results[(b << 2) | (side << 1) | h]["out"])
                 .astype(np.float32) for h in (0, 1)], axis=1)  # [4096, 576]
            contrib = feat.reshape(C, 4, 4, 24, 24)
            acc = np.zeros((C, 50, 50), np.float32)
            for i in range(4):
                for j in range(4):
                    acc[:, i:i + 48:2, j:j + 48:2] += contrib[:, i, j]
            y[b, side] = acc[:, 1:49, 1:49] * 0.25
    j = np.arange(W, dtype=np.float32)
    w = (0.5 * (np.cos(np.pi * j / (W - 1)) + 1.0)).reshape(1, 1, 1, W)
    return w * y[:, 0] + w[..., ::-1] * y[:, 1]


def _run(inputs, trace=False):
    from concourse.bass_utils import run_bass_kernel_spmd

    if "nc" not in _CACHED:
        _CACHED["nc"] = _build_nc()
    in_maps = _prep_inputs(inputs)
    res = run_bass_kernel_spmd(_CACHED["nc"], in_maps, list(range(8)),
                               trace=trace)
    return _postprocess(res.results), res


def kernel(**inputs):
    out, _ = _run(inputs)
    return out
